# revision 27
# baseline (speedup 1.0000x reference)
"""Causal multi-head attention layer on 8 trn2 NeuronCores.

Sharding: 8 cores = 4 batches x 2 head-groups. Core c handles batch c//2 and
heads [8*(c%2), 8*(c%2)+8). Each core runs QKV projections for its 512-wide
head slice, causal flash attention for 8 heads, and a partial output
projection (its 512 rows of Wo). Host sums the two partials per batch + bo.

Schedule: the attention inner loop is PE-bound only if the exp-ACTIVATE
latency is hidden; projection/out-projection matmuls are flattened into
single-matmul "atoms" and woven one per attention step (plus bulk drains at
head-pair boundaries). outproj(1)/(2) are deferred into the last l-chunk,
which has no next-projection to weave.

Problem constants (hardcoded per contract): B=4, L=2048, D=1024, H=16, DK=DV=64.
"""

import sys

import os
for _p in ("/opt/trn_rl_repo", "/root/.axon_site/_ro/trn_rl_repo"):
    if os.path.isdir(_p) and _p not in sys.path:
        sys.path.insert(0, _p)

import numpy as np
import ml_dtypes

import concourse.bass as bass
import concourse.tile as tile
from concourse import bacc, mybir
from concourse.bass_utils import run_bass_kernel_spmd
BF16 = ml_dtypes.bfloat16

B, L, D, H, DK, DV = 4, 2048, 1024, 16, 64, 64
N_CORES = 8
HL = 8          # heads per core
DH = 512        # local head dim (HL * DK)
P = 128
LC = 512        # l-chunk
NLC = L // LC   # 4
NDC = D // P    # 8 contraction chunks for projections
NKC = DH // P   # 4 dk chunks
NST = L // P    # 16 s tiles
VW = DV + 1     # 65: V columns + ones column
SCALE = 1.0 / np.sqrt(DK)
MASK_NEG = -1.0e5

TRACE = False          # set by test harness for profiling runs
LAST_RESULTS = None    # BassKernelResults of the last run (for profiling)

_COMPILED = None


def _build():
    f32 = mybir.dt.float32
    bf16 = mybir.dt.bfloat16
    AF = mybir.ActivationFunctionType

    nc = bacc.Bacc("TRN2", target_bir_lowering=False, debug=False,
                   num_devices=N_CORES)

    xq0 = nc.dram_tensor("xq0", [P, LC], bf16, kind="ExternalInput").ap()
    wq0 = nc.dram_tensor("wq0", [P, DH], bf16, kind="ExternalInput").ap()
    xqR = nc.dram_tensor("xqR", [P, NDC - 1, LC], bf16, kind="ExternalInput").ap()
    wqR = nc.dram_tensor("wqR", [P, NDC - 1, DH], bf16, kind="ExternalInput").ap()
    xqT = nc.dram_tensor("xqT", [NLC, P, NDC, LC], bf16, kind="ExternalInput").ap()
    xkT = nc.dram_tensor("xkT", [NLC, P, NDC, LC], bf16, kind="ExternalInput").ap()
    xvT = nc.dram_tensor("xvT", [NLC, P, NDC, LC], bf16, kind="ExternalInput").ap()
    wq = nc.dram_tensor("wq", [P, NDC, DH], bf16, kind="ExternalInput").ap()
    wk = nc.dram_tensor("wk", [P, NDC, DH], bf16, kind="ExternalInput").ap()
    wv = nc.dram_tensor("wv", [P, NDC, DH], bf16, kind="ExternalInput").ap()
    wo = nc.dram_tensor("wo", [P, NKC, D], bf16, kind="ExternalInput").ap()
    bq = nc.dram_tensor("bq", [P, NKC], f32, kind="ExternalInput").ap()
    bk = nc.dram_tensor("bk", [P, NKC], f32, kind="ExternalInput").ap()
    bv = nc.dram_tensor("bv", [P, DH], f32, kind="ExternalInput").ap()
    outp = nc.dram_tensor("outp", [L, D], f32, kind="ExternalOutput").ap()

    from contextlib import ExitStack

    with tile.TileContext(nc) as tc, ExitStack() as ctx:
        const = ctx.enter_context(tc.tile_pool(name="const", bufs=1))
        kvp = ctx.enter_context(tc.tile_pool(name="kv", bufs=1))
        xp = ctx.enter_context(tc.tile_pool(name="x", bufs=2))
        qp = ctx.enter_context(tc.tile_pool(name="qt", bufs=2))
        ptp = ctx.enter_context(tc.tile_pool(name="pt", bufs=6))
        atp = ctx.enter_context(tc.tile_pool(name="at", bufs=4))
        osb = ctx.enter_context(tc.tile_pool(name="osb", bufs=4))
        nrm = ctx.enter_context(tc.tile_pool(name="nrm", bufs=3))
        ps_pj = ctx.enter_context(tc.tile_pool(name="ps_pj", bufs=2, space="PSUM"))
        ps_s = ctx.enter_context(tc.tile_pool(name="ps_s", bufs=2, space="PSUM"))
        ps_o = ctx.enter_context(tc.tile_pool(name="ps_o", bufs=2, space="PSUM"))

        # ---- constants / first-chunk loads ----
        # Startup is DMA-issue bound (~550ns per dma_start): split the first
        # matmul's deps (wq dc=0 + xq dc=0) onto Sync, the remainders onto
        # the otherwise-idle Scalar/Vector queues, k/v path onto GpSimd.
        # first-chunk deps live in their own tiles: a tile's readers wait on
        # ALL of its writer DMAs, so splitting one tile's load would not help
        wqa = const.tile([P, DH], bf16, tag="wqa")
        wq_sb = const.tile([P, NDC - 1, DH], bf16, tag="wq")
        x0a = const.tile([P, LC], bf16, tag="xq0")
        x0r = const.tile([P, NDC - 1, LC], bf16, tag="xq0r")
        x0 = [None] + [xp.tile([P, NDC, LC], bf16, tag=nm, name=nm)
                       for nm in ("xk", "xv")]
        nc.sync.dma_start(wqa[:], wq0[:])
        nc.sync.dma_start(x0a[:], xq0[:])
        # gate the bulk loads on the first-chunk arrivals: the DMA engines
        # round-robin all rings, so pushing the bulk immediately would starve
        # the two small critical transfers behind ~5MB of descriptors
        gate = const.tile([1, 8], f32, tag="gate")
        nc.scalar.activation(gate[0:1, 0:4], x0a[0:1, 0:4], AF.Copy)
        nc.gpsimd.tensor_copy(gate[0:1, 4:8], x0a[0:1, 4:8])
        nc.scalar.dma_start(wq_sb[:], wqR[:])
        nc.scalar.dma_start(x0r[:], xqR[:])

        # additive causal mask for diagonal 128x128 blocks of S^T (s part, l free)
        cmask = const.tile([P, P], f32, tag="cmask")
        nc.gpsimd.memset(cmask[:], 0.0)
        nc.gpsimd.affine_select(
            out=cmask[:], in_=cmask[:],
            compare_op=mybir.AluOpType.is_ge,
            fill=MASK_NEG, base=0,
            pattern=[[1, P]], channel_multiplier=-1,
        )
        nc.gpsimd.dma_start(x0[1][:], xkT[0])
        nc.gpsimd.dma_start(x0[2][:], xvT[0])
        wk_sb = const.tile([P, NDC, DH], bf16, tag="wk")
        nc.gpsimd.dma_start(wk_sb[:], wk[:])
        wv_sb = const.tile([P, NDC, DH], bf16, tag="wv")
        nc.gpsimd.dma_start(wv_sb[:], wv[:])
        bq_sb = const.tile([P, NKC], f32, tag="bq")
        nc.sync.dma_start(bq_sb[:], bq[:])
        bk_sb = const.tile([P, NKC], f32, tag="bk")
        nc.sync.dma_start(bk_sb[:], bk[:])
        bv_sb = const.tile([P, DH], f32, tag="bv")
        nc.sync.dma_start(bv_sb[:], bv[:])
        wo_sb = const.tile([P, NKC, D], bf16, tag="wo")
        nc.sync.dma_start(wo_sb[:], wo[:])

        # persistent K^T (dk, s) and V (s, dv|1) for the whole core
        kT_sb = kvp.tile([P, NKC, L], bf16, tag="kT")
        v_sb = kvp.tile([P, NST, HL * VW], bf16, tag="v")
        ones_view = v_sb[:].rearrange("p t (h c) -> p t h c", c=VW)[:, :, :, DV:]
        nc.vector.memset(ones_view, 1.0)

        def wqf(dc):
            return wqa[:] if dc == 0 else wq_sb[:, dc - 1, :]
        W = {"q": (wqf, bq_sb), "k": (lambda dc: wk_sb[:, dc, :], bk_sb)}

        def load_x(lc, preloaded=None):
            if preloaded is not None:
                xk_t, xv_t = preloaded[1], preloaded[2]
                return (lambda dc: x0a[:] if dc == 0 else x0r[:, dc - 1, :],
                        lambda dc: xk_t[:, dc, :],
                        lambda dc: xv_t[:, dc, :])
            xs = []
            for nm, dram in (("xq", xqT), ("xk", xkT), ("xv", xvT)):
                t = xp.tile([P, NDC, LC], bf16, tag=nm, name=nm)
                nc.sync.dma_start(t[:], dram[lc])
                xs.append(t)
            return tuple((lambda t: lambda dc: t[:, dc, :])(t) for t in xs)

        # ---- atom builders: each atom is ~one PE matmul (plus cheap tail) ----
        def proj_atoms(lc, xs, qt_t):
            """QKV projection for chunk lc as a flat list of matmul atoms."""
            xq_f, xk_f, xv_f = xs
            atoms = []

            def qk_group(which, kc):
                w_f, b_sb = W[which]
                x_f = xq_f if which == "q" else xk_f
                cell = {}

                def mk(dc):
                    def emit():
                        if dc == 0:
                            cell["ps"] = ps_pj.tile([P, LC], f32, tag="ps_pj",
                                                    name="ps_pj")
                        nc.tensor.matmul(cell["ps"][:],
                                         w_f(dc)[:, bass.ts(kc, P)],
                                         x_f(dc),
                                         start=(dc == 0), stop=(dc == NDC - 1))
                        if dc == NDC - 1:
                            dst = (qt_t[:, kc, :] if which == "q"
                                   else kT_sb[:, kc, bass.ts(lc, LC)])
                            nc.vector.tensor_scalar_add(dst, cell["ps"][:],
                                                        b_sb[:, kc:kc + 1])
                    return emit
                return [mk(dc) for dc in range(NDC)]

            def v_group(j):
                st = lc * (LC // P) + j
                cell = {}

                def mk(dc):
                    def emit():
                        if dc == 0:
                            cell["ps"] = ps_pj.tile([P, LC], f32, tag="ps_pj",
                                                    name="ps_pj")
                        nc.tensor.matmul(cell["ps"][:],
                                         xv_f(dc)[:, bass.ts(j, P)],
                                         wv_sb[:, dc, :],
                                         start=(dc == 0), stop=(dc == NDC - 1))
                        if dc == NDC - 1:
                            vv = v_sb[:, st, :].rearrange(
                                "p (h c) -> p h c", c=VW)[:, :, :DV]
                            nc.vector.tensor_tensor(
                                vv,
                                cell["ps"][:].rearrange("p (h c) -> p h c", c=DV),
                                bv_sb[:].rearrange("p (h c) -> p h c", c=DV),
                                mybir.AluOpType.add)
                    return emit
                return [mk(dc) for dc in range(NDC)]

            for kc in range(NKC):
                atoms += qk_group("q", kc)
            for kc in range(NKC):
                atoms += qk_group("k", kc)
            for j in range(LC // P):
                atoms += v_group(j)
            return [(0, a) for a in atoms]

        def outproj_atoms(lc, at_t):
            """Out-projection for chunk lc as (ready_slot, fn) matmul atoms."""
            atoms = []

            def op_group(lt):
                o_cell = {}
                group = []
                for n in range(2):
                    cell = {}

                    def mk(hc, n=n, cell=cell):
                        def emit():
                            if hc == 0:
                                if "o" not in o_cell:
                                    o_cell["o"] = osb.tile([P, D], f32,
                                                           tag="o_sb", name="o_sb")
                                cell["ps"] = ps_pj.tile([P, LC], f32,
                                                        tag="ps_pj", name="ps_pj")
                            nc.tensor.matmul(cell["ps"][:],
                                             at_t[hc][:, bass.ts(lt, P)],
                                             wo_sb[:, hc, bass.ts(n, 512)],
                                             start=(hc == 0), stop=(hc == NKC - 1))
                            if hc == NKC - 1:
                                nc.vector.tensor_copy(
                                    o_cell["o"][:, bass.ts(n, 512)], cell["ps"][:])
                        return emit
                    group += [(0, mk(hc)) for hc in range(NKC)]

                def flush():
                    nc.sync.dma_start(
                        outp[lc * LC + lt * P: lc * LC + (lt + 1) * P, :],
                        o_cell["o"][:])
                group.append((0, flush))
                return group

            for lt in range(LC // P):
                atoms += op_group(lt)
            return atoms

        def outproj_split_atoms(lc, at_t, n_st):
            """Out-projection for the chunk whose attention is still running:
            two-hc psum chains gated by ready_slot so each matmul emits only
            after the normalize that produces its at_t half. Tail = only the
            hc2+hc3 chains' last matmuls + adds."""
            o_cells = [{} for _ in range(LC // P)]
            halves = []
            # chain groups: (hc0+hc1 -> copy) ready when at1 lands, then
            # single-hc add chains for hc2 and hc3; whole-chain ready values
            # so FIFO gating never leaves a psum chain half-open
            for hcs in ((0, 1), (2,), (3,)):
                ready = (hcs[-1] + 1) * n_st
                for lt in range(LC // P):
                    for n in range(2):
                        cell = {}
                        for j, hc in enumerate(hcs):
                            last = j == len(hcs) - 1

                            def emit(j=j, hc=hc, n=n, lt=lt, cell=cell,
                                     first=(hcs[0] == 0), last=last):
                                if j == 0:
                                    if "o" not in o_cells[lt]:
                                        o_cells[lt]["o"] = osb.tile(
                                            [P, D], f32, tag="o_sb", name="o_sb")
                                    cell["ps"] = ps_pj.tile([P, LC], f32,
                                                            tag="ps_pj", name="ps_pj")
                                nc.tensor.matmul(cell["ps"][:],
                                                 at_t[hc][:, bass.ts(lt, P)],
                                                 wo_sb[:, hc, bass.ts(n, 512)],
                                                 start=(j == 0), stop=last)
                                if last:
                                    dst = o_cells[lt]["o"][:, bass.ts(n, 512)]
                                    if first:
                                        nc.vector.tensor_copy(dst, cell["ps"][:])
                                    else:
                                        nc.vector.tensor_tensor(
                                            dst, dst, cell["ps"][:],
                                            mybir.AluOpType.add)
                            halves.append((ready, emit))

            def mk_flush(lt):
                def flush():
                    nc.sync.dma_start(
                        outp[lc * LC + lt * P: lc * LC + (lt + 1) * P, :],
                        o_cells[lt]["o"][:])
                return flush
            for lt in range(LC // P):
                halves.append((4 * n_st, mk_flush(lt)))
            return halves

        def attention(lc, qt_t, atoms, at_t):
            """Causal attention for chunk lc as a flat (pair, step) pipeline:
            scores+exp issue one slot ahead (across pair boundaries, so the
            scalar engine stays fed through the normalize/drain regions); one
            filler atom weaves per slot when its ready_slot allows, surplus
            drains at pair boundaries."""
            n_st = (lc + 1) * (LC // P)

            def mm1(hp, st):
                # S^T: two heads packed on PE row halves, one 2-bank psum
                jj = st - lc * (LC // P)
                nc0 = jj * P if jj >= 0 else 0
                s01 = ps_s.tile([P, 2 * LC], f32, tag="ps_s", name="ps_s")
                nc.tensor.matmul(s01[:, nc0:LC], kT_sb[0:64, hp, bass.ts(st, P)],
                                 qt_t[0:64, hp, nc0:], start=True, stop=True,
                                 tile_position=(0, 0))
                nc.tensor.matmul(s01[:, LC + nc0:], kT_sb[64:128, hp, bass.ts(st, P)],
                                 qt_t[64:128, hp, nc0:], start=True, stop=True,
                                 tile_position=(64, 0))
                if jj >= 0:
                    dview = s01[:].rearrange("p (t c) -> p t c", t=2)[:, :, nc0:nc0 + P]
                    nc.vector.tensor_tensor(
                        dview, dview,
                        cmask[:, None, :].to_broadcast([P, 2, P]),
                        mybir.AluOpType.add)
                return s01, nc0

            def normalize(hp, po0, po1):
                # evict psum, then normalize off-psum (h1 first: longer chain)
                un0 = nrm.tile([P, LC], f32, tag="un", name="un")
                nc.vector.tensor_copy(un0[0:VW, :], po0[0:VW, :])
                un1 = nrm.tile([P, LC], f32, tag="un", name="un")
                nc.vector.tensor_copy(un1[0:VW, :], po1[0:VW, :])

                rz1 = nrm.tile([1, LC], f32, tag="rz", name="rz")
                nc.sync.dma_start(rz1[:], un1[64:65, :])
                rr1 = nrm.tile([1, LC], f32, tag="rr", name="rr")
                nc.vector.reciprocal_approx_fast(rr1[:], rz1[:])
                rb1 = nrm.tile([64, LC], f32, tag="rb", name="rb")
                nc.gpsimd.partition_broadcast(rb1[:], rr1[:])
                tmp1 = nrm.tile([64, LC], bf16, tag="tmp1", name="tmp1")
                nc.vector.tensor_mul(tmp1[:], un1[0:64, :], rb1[:])
                nc.sync.dma_start(at_t[hp][64:128, :], tmp1[:])

                rz0 = nrm.tile([1, LC], f32, tag="rz", name="rz")
                nc.sync.dma_start(rz0[:], un0[64:65, :])
                rr0 = nrm.tile([1, LC], f32, tag="rr", name="rr")
                nc.vector.reciprocal_approx_fast(rr0[:], rz0[:])
                rb0 = nrm.tile([64, LC], f32, tag="rb", name="rb")
                nc.gpsimd.partition_broadcast(rb0[:], rr0[:])
                nc.vector.tensor_mul(at_t[hp][0:64, :], un0[0:64, :], rb0[:])

            slots = [(hp, st) for hp in range(NKC) for st in range(n_st)]
            po = {}
            pend = mm1(*slots[0])
            for i, (hp, st) in enumerate(slots):
                s01, nc0 = pend
                if i + 1 < len(slots):
                    pend = mm1(*slots[i + 1])  # PE one slot ahead of ACT
                pt01 = ptp.tile([P, 2 * LC], bf16, tag="pt", name="pt")
                nc.scalar.activation(
                    pt01[:].rearrange("p (t c) -> p t c", t=2)[:, :, nc0:],
                    s01[:].rearrange("p (t c) -> p t c", t=2)[:, :, nc0:],
                    AF.Exp, bias=0.0, scale=float(SCALE))
                if atoms and atoms[0][0] <= i:
                    atoms.pop(0)[1]()  # PE filler for the ACT latency
                if st == 0:
                    po[hp] = (ps_o.tile([P, LC], f32, tag="ps_o", name="ps_o"),
                              ps_o.tile([P, LC], f32, tag="ps_o", name="ps_o"))
                po0, po1 = po[hp]
                h0, h1 = 2 * hp, 2 * hp + 1
                nc.tensor.matmul(po0[0:VW, nc0:], v_sb[:, st, h0 * VW:(h0 + 1) * VW],
                                 pt01[:, nc0:LC],
                                 start=(st == 0), stop=(st == n_st - 1))
                nc.tensor.matmul(po1[0:VW, nc0:], v_sb[:, st, h1 * VW:(h1 + 1) * VW],
                                 pt01[:, LC + nc0:],
                                 start=(st == 0), stop=(st == n_st - 1))
                if st == n_st - 1:
                    normalize(hp, po0, po1)
                    # bulk-drain surplus atoms, keeping a reserve to cover
                    # the final pair's normalize latency with PE work
                    slots_left = len(slots) - 1 - i
                    bounds_left = (slots_left + n_st - 1) // n_st
                    if bounds_left:
                        surplus = len(atoms) - slots_left - 18
                        b = -(-surplus // bounds_left) if surplus > 0 else 0
                        for _ in range(b):
                            if not atoms or atoms[0][0] > i:
                                break
                            atoms.pop(0)[1]()
            while atoms:
                atoms.pop(0)[1]()
            return at_t

        # ---- pipeline ----
        # proj(0) runs alone; attention(lc) weaves: lc=0 -> proj(1),
        # lc=1 -> proj(2)+outproj(0), lc=2 -> proj(3), lc=3 -> outproj(1)+(2)
        # +outproj(3) (split chains gated by ready_slot, so only the hc2+hc3
        # chains' final matmuls trail the last normalize).
        xs = load_x(0, preloaded=x0)
        qt0 = qp.tile([P, NKC, LC], bf16, tag="qt", name="qt")
        for _, a in proj_atoms(0, xs, qt0):
            a()
        qt_cur = qt0
        at_all = []
        for lc in range(NLC):
            at_t = [atp.tile([P, LC], bf16, tag=f"at{hc}", name=f"at{hc}")
                    for hc in range(NKC)]
            at_all.append(at_t)
            atoms = []
            if lc + 1 < NLC:
                xs = load_x(lc + 1)
                qt_nxt = qp.tile([P, NKC, LC], bf16, tag="qt", name="qt")
                atoms += proj_atoms(lc + 1, xs, qt_nxt)
            if lc == 1:
                atoms += outproj_atoms(0, at_all[0])
            if lc == 3:
                atoms += outproj_atoms(1, at_all[1])
                atoms += outproj_atoms(2, at_all[2])
                atoms += outproj_split_atoms(3, at_t, (lc + 1) * (LC // P))
            attention(lc, qt_cur, atoms, at_t)
            if lc + 1 < NLC:
                qt_cur = qt_nxt

    nc.compile()
    return nc


def _get_compiled():
    global _COMPILED
    if _COMPILED is None:
        _COMPILED = _build()
    return _COMPILED


def kernel(queries, keys, values, Wq, bq, Wk, bk, Wv, bv, Wo, bo):
    global LAST_RESULTS
    nc = _get_compiled()

    queries = np.asarray(queries, np.float32)
    keys = np.asarray(keys, np.float32)
    values = np.asarray(values, np.float32)

    def pack_x(x):
        # (L, D) -> (NLC, P, NDC, LC): [lc, p, dc, l] = x[lc*LC+l, dc*P+p]
        t = x.T.reshape(NDC, P, NLC, LC)          # [dc, p, lc, l]
        return np.ascontiguousarray(t.transpose(2, 1, 0, 3)).astype(BF16)

    xT = {}
    for b in range(B):
        xT[("q", b)] = pack_x(np.asarray(queries[b]))
        xT[("k", b)] = pack_x(np.asarray(keys[b]))
        xT[("v", b)] = pack_x(np.asarray(values[b]))

    wslice = {}
    for g in range(2):
        sl = slice(DH * g, DH * (g + 1))
        def pack_w(w):
            # (D, DH) -> (P, NDC, DH)
            return np.ascontiguousarray(
                w.reshape(NDC, P, DH).transpose(1, 0, 2)).astype(BF16)

        wslice[("wq", g)] = pack_w(np.asarray(Wq, np.float32)[:, sl])
        wslice[("wk", g)] = pack_w(np.asarray(Wk, np.float32)[:, sl])
        wslice[("wv", g)] = pack_w(np.asarray(Wv, np.float32)[:, sl])
        wslice[("wo", g)] = np.ascontiguousarray(
            np.asarray(Wo, np.float32)[sl, :].reshape(NKC, P, D).transpose(1, 0, 2)
        ).astype(BF16)
        wslice[("bq", g)] = np.ascontiguousarray(
            np.asarray(bq, np.float32)[sl].reshape(NKC, P).T)
        wslice[("bk", g)] = np.ascontiguousarray(
            np.asarray(bk, np.float32)[sl].reshape(NKC, P).T)
        wslice[("bv", g)] = np.ascontiguousarray(
            np.broadcast_to(np.asarray(bv, np.float32)[sl], (P, DH)))

    in_maps = []
    for c in range(N_CORES):
        b, g = c // 2, c % 2
        in_maps.append({
            "xq0": np.ascontiguousarray(xT[("q", b)][0, :, 0, :]),
            "wq0": np.ascontiguousarray(wslice[("wq", g)][:, 0]),
            "xqR": np.ascontiguousarray(xT[("q", b)][0, :, 1:, :]),
            "wqR": np.ascontiguousarray(wslice[("wq", g)][:, 1:]),
            "xqT": xT[("q", b)], "xkT": xT[("k", b)], "xvT": xT[("v", b)],
            "wq": wslice[("wq", g)], "wk": wslice[("wk", g)],
            "wv": wslice[("wv", g)], "wo": wslice[("wo", g)],
            "bq": wslice[("bq", g)], "bk": wslice[("bk", g)],
            "bv": wslice[("bv", g)],
        })

    res = run_bass_kernel_spmd(nc, in_maps, list(range(N_CORES)), trace=TRACE)
    LAST_RESULTS = res

    bo32 = np.asarray(bo, np.float32)
    out = np.empty((B, L, D), np.float32)
    for b in range(B):
        out[b] = res.results[2 * b]["outp"] + res.results[2 * b + 1]["outp"] + bo32
    return out


# revision 37
# speedup vs baseline: 1.0093x; 1.0093x over previous
"""Causal multi-head attention layer on 8 trn2 NeuronCores.

Sharding: 8 cores = 4 batches x 2 head-groups. Core c handles batch c//2 and
heads [8*(c%2), 8*(c%2)+8). Each core runs QKV projections for its 512-wide
head slice, causal flash attention for 8 heads, and a partial output
projection (its 512 rows of Wo). Host sums the two partials per batch + bo.

Schedule: the attention inner loop is PE-bound only if the exp-ACTIVATE
latency is hidden; projection/out-projection matmuls are flattened into
single-matmul "atoms" and woven one per attention step (plus bulk drains at
head-pair boundaries). outproj(1)/(2) are deferred into the last l-chunk,
which has no next-projection to weave.

Problem constants (hardcoded per contract): B=4, L=2048, D=1024, H=16, DK=DV=64.
"""

import sys

import os
for _p in ("/opt/trn_rl_repo", "/root/.axon_site/_ro/trn_rl_repo"):
    if os.path.isdir(_p) and _p not in sys.path:
        sys.path.insert(0, _p)

import numpy as np
import ml_dtypes

import concourse.bass as bass
import concourse.tile as tile
from concourse import bacc, mybir
from concourse.bass_utils import run_bass_kernel_spmd
BF16 = ml_dtypes.bfloat16

B, L, D, H, DK, DV = 4, 2048, 1024, 16, 64, 64
N_CORES = 8
HL = 8          # heads per core
DH = 512        # local head dim (HL * DK)
P = 128
LC = 512        # l-chunk
NLC = L // LC   # 4
NDC = D // P    # 8 contraction chunks for projections
NKC = DH // P   # 4 dk chunks
NST = L // P    # 16 s tiles
VW = DV + 1     # 65: V columns + ones column
SCALE = 1.0 / np.sqrt(DK)
MASK_NEG = -1.0e5

TRACE = False          # set by test harness for profiling runs
LAST_RESULTS = None    # BassKernelResults of the last run (for profiling)

_COMPILED = None


def _build():
    f32 = mybir.dt.float32
    bf16 = mybir.dt.bfloat16
    AF = mybir.ActivationFunctionType

    nc = bacc.Bacc("TRN2", target_bir_lowering=False, debug=False,
                   num_devices=N_CORES)

    xq0 = nc.dram_tensor("xq0", [P, LC], bf16, kind="ExternalInput").ap()
    wq0 = nc.dram_tensor("wq0", [P, DH], bf16, kind="ExternalInput").ap()
    xqR = nc.dram_tensor("xqR", [P, NDC - 1, LC], bf16, kind="ExternalInput").ap()
    wqR = nc.dram_tensor("wqR", [P, NDC - 1, DH], bf16, kind="ExternalInput").ap()
    xqT = nc.dram_tensor("xqT", [NLC, P, NDC, LC], bf16, kind="ExternalInput").ap()
    xkT = nc.dram_tensor("xkT", [NLC, P, NDC, LC], bf16, kind="ExternalInput").ap()
    xvT = nc.dram_tensor("xvT", [NLC, P, NDC, LC], bf16, kind="ExternalInput").ap()
    wq = nc.dram_tensor("wq", [P, NDC, DH], bf16, kind="ExternalInput").ap()
    wk = nc.dram_tensor("wk", [P, NDC, DH], bf16, kind="ExternalInput").ap()
    wv = nc.dram_tensor("wv", [P, NDC, DH], bf16, kind="ExternalInput").ap()
    wo = nc.dram_tensor("wo", [P, NKC, D], bf16, kind="ExternalInput").ap()
    bq = nc.dram_tensor("bq", [P, NKC], f32, kind="ExternalInput").ap()
    bk = nc.dram_tensor("bk", [P, NKC], f32, kind="ExternalInput").ap()
    bv = nc.dram_tensor("bv", [P, DH], f32, kind="ExternalInput").ap()
    outp = nc.dram_tensor("outp", [L, D], f32, kind="ExternalOutput").ap()

    from contextlib import ExitStack

    with tile.TileContext(nc) as tc, ExitStack() as ctx:
        const = ctx.enter_context(tc.tile_pool(name="const", bufs=1))
        kvp = ctx.enter_context(tc.tile_pool(name="kv", bufs=1))
        xp = ctx.enter_context(tc.tile_pool(name="x", bufs=2))
        qp = ctx.enter_context(tc.tile_pool(name="qt", bufs=2))
        ptp = ctx.enter_context(tc.tile_pool(name="pt", bufs=6))
        atp = ctx.enter_context(tc.tile_pool(name="at", bufs=4))
        osb = ctx.enter_context(tc.tile_pool(name="osb", bufs=5))
        nrm = ctx.enter_context(tc.tile_pool(name="nrm", bufs=3))
        ps_pj = ctx.enter_context(tc.tile_pool(name="ps_pj", bufs=2, space="PSUM"))
        ps_s = ctx.enter_context(tc.tile_pool(name="ps_s", bufs=2, space="PSUM"))
        ps_o = ctx.enter_context(tc.tile_pool(name="ps_o", bufs=2, space="PSUM"))

        # ---- constants / first-chunk loads ----
        # Startup is DMA-issue bound (~550ns per dma_start): split the first
        # matmul's deps (wq dc=0 + xq dc=0) onto Sync, the remainders onto
        # the otherwise-idle Scalar/Vector queues, k/v path onto GpSimd.
        # first-chunk deps live in their own tiles: a tile's readers wait on
        # ALL of its writer DMAs, so splitting one tile's load would not help
        wqa = const.tile([P, DH], bf16, tag="wqa")
        wq_sb = const.tile([P, NDC - 1, DH], bf16, tag="wq")
        x0a = const.tile([P, LC], bf16, tag="xq0")
        x0r = const.tile([P, NDC - 1, LC], bf16, tag="xq0r")
        x0 = [None] + [xp.tile([P, NDC, LC], bf16, tag=nm, name=nm)
                       for nm in ("xk", "xv")]
        nc.sync.dma_start(wqa[:], wq0[:])
        nc.sync.dma_start(x0a[:], xq0[:])
        nc.scalar.dma_start(wq_sb[:], wqR[:])
        nc.scalar.dma_start(x0r[:], xqR[:])

        # additive causal mask for diagonal 128x128 blocks of S^T (s part, l free)
        cmask = const.tile([P, P], f32, tag="cmask")
        nc.gpsimd.memset(cmask[:], 0.0)
        nc.gpsimd.affine_select(
            out=cmask[:], in_=cmask[:],
            compare_op=mybir.AluOpType.is_ge,
            fill=MASK_NEG, base=0,
            pattern=[[1, P]], channel_multiplier=-1,
        )
        nc.gpsimd.dma_start(x0[1][:], xkT[0])
        nc.gpsimd.dma_start(x0[2][:], xvT[0])
        wk_sb = const.tile([P, NDC, DH], bf16, tag="wk")
        nc.gpsimd.dma_start(wk_sb[:], wk[:])
        wv_sb = const.tile([P, NDC, DH], bf16, tag="wv")
        nc.gpsimd.dma_start(wv_sb[:], wv[:])
        bq_sb = const.tile([P, NKC], f32, tag="bq")
        nc.sync.dma_start(bq_sb[:], bq[:])
        bk_sb = const.tile([P, NKC], f32, tag="bk")
        nc.sync.dma_start(bk_sb[:], bk[:])
        bv_sb = const.tile([P, DH], f32, tag="bv")
        nc.sync.dma_start(bv_sb[:], bv[:])
        wo_sb = const.tile([P, NKC, D], bf16, tag="wo")
        nc.sync.dma_start(wo_sb[:], wo[:])

        # persistent K^T (dk, s) and V (s, dv|1) for the whole core
        kT_sb = kvp.tile([P, NKC, L], bf16, tag="kT")
        v_sb = kvp.tile([P, NST, HL * VW], bf16, tag="v")
        ones_view = v_sb[:].rearrange("p t (h c) -> p t h c", c=VW)[:, :, :, DV:]
        nc.vector.memset(ones_view, 1.0)

        def wqf(dc):
            return wqa[:] if dc == 0 else wq_sb[:, dc - 1, :]
        W = {"q": (wqf, bq_sb), "k": (lambda dc: wk_sb[:, dc, :], bk_sb)}

        def load_x(lc, preloaded=None):
            if preloaded is not None:
                xk_t, xv_t = preloaded[1], preloaded[2]
                return (lambda dc: x0a[:] if dc == 0 else x0r[:, dc - 1, :],
                        lambda dc: xk_t[:, dc, :],
                        lambda dc: xv_t[:, dc, :])
            xs = []
            for nm, dram in (("xq", xqT), ("xk", xkT), ("xv", xvT)):
                t = xp.tile([P, NDC, LC], bf16, tag=nm, name=nm)
                nc.sync.dma_start(t[:], dram[lc])
                xs.append(t)
            return tuple((lambda t: lambda dc: t[:, dc, :])(t) for t in xs)

        # ---- atom builders: each atom is ~one PE matmul (plus cheap tail) ----
        def proj_atoms(lc, xs, qt_t):
            """QKV projection for chunk lc as a flat list of matmul atoms."""
            xq_f, xk_f, xv_f = xs
            atoms = []

            def qk_group(which, kc):
                w_f, b_sb = W[which]
                x_f = xq_f if which == "q" else xk_f
                cell = {}

                def mk(dc):
                    def emit():
                        if dc == 0:
                            cell["ps"] = ps_pj.tile([P, LC], f32, tag="ps_pj",
                                                    name="ps_pj")
                        nc.tensor.matmul(cell["ps"][:],
                                         w_f(dc)[:, bass.ts(kc, P)],
                                         x_f(dc),
                                         start=(dc == 0), stop=(dc == NDC - 1))
                        if dc == NDC - 1:
                            dst = (qt_t[:, kc, :] if which == "q"
                                   else kT_sb[:, kc, bass.ts(lc, LC)])
                            nc.vector.tensor_scalar_add(dst, cell["ps"][:],
                                                        b_sb[:, kc:kc + 1])
                    return emit
                return [mk(dc) for dc in range(NDC)]

            def v_group(j):
                st = lc * (LC // P) + j
                cell = {}

                def mk(dc):
                    def emit():
                        if dc == 0:
                            cell["ps"] = ps_pj.tile([P, LC], f32, tag="ps_pj",
                                                    name="ps_pj")
                        nc.tensor.matmul(cell["ps"][:],
                                         xv_f(dc)[:, bass.ts(j, P)],
                                         wv_sb[:, dc, :],
                                         start=(dc == 0), stop=(dc == NDC - 1))
                        if dc == NDC - 1:
                            vv = v_sb[:, st, :].rearrange(
                                "p (h c) -> p h c", c=VW)[:, :, :DV]
                            nc.vector.tensor_tensor(
                                vv,
                                cell["ps"][:].rearrange("p (h c) -> p h c", c=DV),
                                bv_sb[:].rearrange("p (h c) -> p h c", c=DV),
                                mybir.AluOpType.add)
                    return emit
                return [mk(dc) for dc in range(NDC)]

            for kc in range(NKC):
                atoms += qk_group("q", kc)
            for kc in range(NKC):
                atoms += qk_group("k", kc)
            for j in range(LC // P):
                atoms += v_group(j)
            return [(0, a) for a in atoms]

        def outproj_atoms(lc, at_t):
            """Out-projection for chunk lc as (ready_slot, fn) matmul atoms."""
            atoms = []

            def op_group(lt):
                o_cell = {}
                group = []
                for n in range(2):
                    cell = {}

                    def mk(hc, n=n, cell=cell):
                        def emit():
                            if hc == 0:
                                if "o" not in o_cell:
                                    o_cell["o"] = osb.tile([P, D], f32,
                                                           tag="o_sb", name="o_sb")
                                cell["ps"] = ps_pj.tile([P, LC], f32,
                                                        tag="ps_pj", name="ps_pj")
                            nc.tensor.matmul(cell["ps"][:],
                                             at_t[hc][:, bass.ts(lt, P)],
                                             wo_sb[:, hc, bass.ts(n, 512)],
                                             start=(hc == 0), stop=(hc == NKC - 1))
                            if hc == NKC - 1:
                                nc.vector.tensor_copy(
                                    o_cell["o"][:, bass.ts(n, 512)], cell["ps"][:])
                        return emit
                    group += [(0, mk(hc)) for hc in range(NKC)]

                def flush():
                    nc.sync.dma_start(
                        outp[lc * LC + lt * P: lc * LC + (lt + 1) * P, :],
                        o_cell["o"][:])
                group.append((0, flush))
                return group

            for lt in range(LC // P):
                atoms += op_group(lt)
            return atoms

        def outproj_split_atoms(lc, at_t, n_st):
            """Out-projection for the chunk whose attention is still running:
            two-hc psum chains gated by ready_slot so each matmul emits only
            after the normalize that produces its at_t half. Tail = only the
            hc2+hc3 chains' last matmuls + adds."""
            o_cells = [{} for _ in range(LC // P)]
            halves = []
            # chain groups: (hc0+hc1 -> copy) ready when at1 lands, then
            # single-hc add chains for hc2 and hc3; whole-chain ready values
            # so FIFO gating never leaves a psum chain half-open
            for hcs in ((0, 1), (2,), (3,)):
                ready = (hcs[-1] + 1) * n_st
                for lt in range(LC // P):
                    for n in range(2):
                        cell = {}
                        for j, hc in enumerate(hcs):
                            last = j == len(hcs) - 1

                            def emit(j=j, hc=hc, n=n, lt=lt, cell=cell,
                                     first=(hcs[0] == 0), last=last):
                                if j == 0:
                                    if "o" not in o_cells[lt]:
                                        o_cells[lt]["o"] = osb.tile(
                                            [P, D], f32, tag="o_sb", name="o_sb")
                                    cell["ps"] = ps_pj.tile([P, LC], f32,
                                                            tag="ps_pj", name="ps_pj")
                                nc.tensor.matmul(cell["ps"][:],
                                                 at_t[hc][:, bass.ts(lt, P)],
                                                 wo_sb[:, hc, bass.ts(n, 512)],
                                                 start=(j == 0), stop=last)
                                if last:
                                    dst = o_cells[lt]["o"][:, bass.ts(n, 512)]
                                    if first:
                                        nc.vector.tensor_copy(dst, cell["ps"][:])
                                    else:
                                        nc.vector.tensor_tensor(
                                            dst, dst, cell["ps"][:],
                                            mybir.AluOpType.add)
                            halves.append((ready, emit))

            def mk_flush(lt):
                def flush():
                    nc.sync.dma_start(
                        outp[lc * LC + lt * P: lc * LC + (lt + 1) * P, :],
                        o_cells[lt]["o"][:])
                return flush
            for lt in range(LC // P):
                halves.append((4 * n_st, mk_flush(lt)))
            return halves

        def attention(lc, qt_t, atoms, at_t):
            """Causal attention for chunk lc as a flat (pair, step) pipeline:
            scores+exp issue one slot ahead (across pair boundaries, so the
            scalar engine stays fed through the normalize/drain regions); one
            filler atom weaves per slot when its ready_slot allows, surplus
            drains at pair boundaries."""
            n_st = (lc + 1) * (LC // P)

            def mm1(hp, st):
                # S^T: two heads packed on PE row halves, one 2-bank psum
                jj = st - lc * (LC // P)
                nc0 = jj * P if jj >= 0 else 0
                s01 = ps_s.tile([P, 2 * LC], f32, tag="ps_s", name="ps_s")
                nc.tensor.matmul(s01[:, nc0:LC], kT_sb[0:64, hp, bass.ts(st, P)],
                                 qt_t[0:64, hp, nc0:], start=True, stop=True,
                                 tile_position=(0, 0))
                nc.tensor.matmul(s01[:, LC + nc0:], kT_sb[64:128, hp, bass.ts(st, P)],
                                 qt_t[64:128, hp, nc0:], start=True, stop=True,
                                 tile_position=(64, 0))
                if jj >= 0:
                    dview = s01[:].rearrange("p (t c) -> p t c", t=2)[:, :, nc0:nc0 + P]
                    nc.vector.tensor_tensor(
                        dview, dview,
                        cmask[:, None, :].to_broadcast([P, 2, P]),
                        mybir.AluOpType.add)
                return s01, nc0

            def normalize(hp, po0, po1):
                # evict the denominator rows first (tiny copies) so their DMA
                # hop starts ~1.2us earlier, then the bulk psum evictions
                un0 = nrm.tile([P, LC], f32, tag="un", name="un")
                un1 = nrm.tile([P, LC], f32, tag="un", name="un")
                nc.vector.tensor_copy(un1[64:65, :], po1[64:65, :])
                nc.vector.tensor_copy(un0[64:65, :], po0[64:65, :])
                nc.vector.tensor_copy(un0[0:64, :], po0[0:64, :])
                nc.vector.tensor_copy(un1[0:64, :], po1[0:64, :])

                rz1 = nrm.tile([1, LC], f32, tag="rz", name="rz")
                nc.sync.dma_start(rz1[:], un1[64:65, :])
                rz0 = nrm.tile([1, LC], f32, tag="rz", name="rz")
                nc.sync.dma_start(rz0[:], un0[64:65, :])
                rr1 = nrm.tile([1, LC], f32, tag="rr", name="rr")
                nc.vector.reciprocal_approx_fast(rr1[:], rz1[:])
                rr0 = nrm.tile([1, LC], f32, tag="rr", name="rr")
                nc.vector.reciprocal_approx_fast(rr0[:], rz0[:])
                rb1 = nrm.tile([64, LC], f32, tag="rb", name="rb")
                nc.gpsimd.partition_broadcast(rb1[:], rr1[:])
                rb0 = nrm.tile([64, LC], f32, tag="rb", name="rb")
                nc.gpsimd.partition_broadcast(rb0[:], rr0[:])
                tmp1 = nrm.tile([64, LC], bf16, tag="tmp1", name="tmp1")
                nc.vector.tensor_mul(tmp1[:], un1[0:64, :], rb1[:])
                nc.sync.dma_start(at_t[hp][64:128, :], tmp1[:])
                nc.vector.tensor_mul(at_t[hp][0:64, :], un0[0:64, :], rb0[:])

            slots = [(hp, st) for hp in range(NKC) for st in range(n_st)]
            po = {}
            pend = mm1(*slots[0])
            for i, (hp, st) in enumerate(slots):
                s01, nc0 = pend
                if i + 1 < len(slots):
                    pend = mm1(*slots[i + 1])  # PE one slot ahead of ACT
                pt01 = ptp.tile([P, 2 * LC], bf16, tag="pt", name="pt")
                nc.scalar.activation(
                    pt01[:].rearrange("p (t c) -> p t c", t=2)[:, :, nc0:],
                    s01[:].rearrange("p (t c) -> p t c", t=2)[:, :, nc0:],
                    AF.Exp, bias=0.0, scale=float(SCALE))
                if atoms and atoms[0][0] <= i:
                    atoms.pop(0)[1]()  # PE filler for the ACT latency
                if st == 0:
                    po[hp] = (ps_o.tile([P, LC], f32, tag="ps_o", name="ps_o"),
                              ps_o.tile([P, LC], f32, tag="ps_o", name="ps_o"))
                po0, po1 = po[hp]
                h0, h1 = 2 * hp, 2 * hp + 1
                nc.tensor.matmul(po0[0:VW, nc0:], v_sb[:, st, h0 * VW:(h0 + 1) * VW],
                                 pt01[:, nc0:LC],
                                 start=(st == 0), stop=(st == n_st - 1))
                nc.tensor.matmul(po1[0:VW, nc0:], v_sb[:, st, h1 * VW:(h1 + 1) * VW],
                                 pt01[:, LC + nc0:],
                                 start=(st == 0), stop=(st == n_st - 1))
                if st == n_st - 1:
                    normalize(hp, po0, po1)
                    # bulk-drain surplus atoms, keeping a reserve to cover
                    # the final pair's normalize latency with PE work
                    slots_left = len(slots) - 1 - i
                    bounds_left = (slots_left + n_st - 1) // n_st
                    if bounds_left:
                        surplus = len(atoms) - slots_left - 18
                        b = -(-surplus // bounds_left) if surplus > 0 else 0
                        for _ in range(b):
                            if not atoms or atoms[0][0] > i:
                                break
                            atoms.pop(0)[1]()
            while atoms:
                atoms.pop(0)[1]()
            return at_t

        # ---- pipeline ----
        # proj(0) runs alone; attention(lc) weaves: lc=0 -> proj(1),
        # lc=1 -> proj(2)+outproj(0), lc=2 -> proj(3), lc=3 -> outproj(1)+(2)
        # +outproj(3) (split chains gated by ready_slot, so only the hc2+hc3
        # chains' final matmuls trail the last normalize).
        xs = load_x(0, preloaded=x0)
        qt0 = qp.tile([P, NKC, LC], bf16, tag="qt", name="qt")
        for _, a in proj_atoms(0, xs, qt0):
            a()
        qt_cur = qt0
        at_all = []
        for lc in range(NLC):
            at_t = [atp.tile([P, LC], bf16, tag=f"at{hc}", name=f"at{hc}")
                    for hc in range(NKC)]
            at_all.append(at_t)
            atoms = []
            if lc + 1 < NLC:
                xs = load_x(lc + 1)
                qt_nxt = qp.tile([P, NKC, LC], bf16, tag="qt", name="qt")
                atoms += proj_atoms(lc + 1, xs, qt_nxt)
            if lc == 1:
                atoms += outproj_atoms(0, at_all[0])
            if lc == 3:
                n_st3 = (lc + 1) * (LC // P)
                a2 = outproj_atoms(2, at_all[2])
                sp = outproj_split_atoms(3, at_t, n_st3)
                # hold 3 outproj(2) groups for the final drain, placed before
                # the at3-gated section: they execute on the PE while the last
                # pair's normalize chain runs, instead of idling behind it
                reserve = [(4 * n_st3, fn) for _, fn in a2[9:]]
                gated = [a for a in sp if a[0] > 3 * n_st3]
                atoms += outproj_atoms(1, at_all[1])
                atoms += a2[:9]
                atoms += [a for a in sp if a[0] <= 3 * n_st3]
                atoms += reserve + gated
            attention(lc, qt_cur, atoms, at_t)
            if lc + 1 < NLC:
                qt_cur = qt_nxt

    nc.compile()
    return nc


def _get_compiled():
    global _COMPILED
    if _COMPILED is None:
        _COMPILED = _build()
    return _COMPILED


def kernel(queries, keys, values, Wq, bq, Wk, bk, Wv, bv, Wo, bo):
    global LAST_RESULTS
    nc = _get_compiled()

    queries = np.asarray(queries, np.float32)
    keys = np.asarray(keys, np.float32)
    values = np.asarray(values, np.float32)

    def pack_x(x):
        # (L, D) -> (NLC, P, NDC, LC): [lc, p, dc, l] = x[lc*LC+l, dc*P+p]
        t = x.T.reshape(NDC, P, NLC, LC)          # [dc, p, lc, l]
        return np.ascontiguousarray(t.transpose(2, 1, 0, 3)).astype(BF16)

    xT = {}
    for b in range(B):
        xT[("q", b)] = pack_x(np.asarray(queries[b]))
        xT[("k", b)] = pack_x(np.asarray(keys[b]))
        xT[("v", b)] = pack_x(np.asarray(values[b]))

    wslice = {}
    for g in range(2):
        sl = slice(DH * g, DH * (g + 1))
        def pack_w(w):
            # (D, DH) -> (P, NDC, DH)
            return np.ascontiguousarray(
                w.reshape(NDC, P, DH).transpose(1, 0, 2)).astype(BF16)

        wslice[("wq", g)] = pack_w(np.asarray(Wq, np.float32)[:, sl])
        wslice[("wk", g)] = pack_w(np.asarray(Wk, np.float32)[:, sl])
        wslice[("wv", g)] = pack_w(np.asarray(Wv, np.float32)[:, sl])
        wslice[("wo", g)] = np.ascontiguousarray(
            np.asarray(Wo, np.float32)[sl, :].reshape(NKC, P, D).transpose(1, 0, 2)
        ).astype(BF16)
        wslice[("bq", g)] = np.ascontiguousarray(
            np.asarray(bq, np.float32)[sl].reshape(NKC, P).T)
        wslice[("bk", g)] = np.ascontiguousarray(
            np.asarray(bk, np.float32)[sl].reshape(NKC, P).T)
        wslice[("bv", g)] = np.ascontiguousarray(
            np.broadcast_to(np.asarray(bv, np.float32)[sl], (P, DH)))

    in_maps = []
    for c in range(N_CORES):
        b, g = c // 2, c % 2
        in_maps.append({
            "xq0": np.ascontiguousarray(xT[("q", b)][0, :, 0, :]),
            "wq0": np.ascontiguousarray(wslice[("wq", g)][:, 0]),
            "xqR": np.ascontiguousarray(xT[("q", b)][0, :, 1:, :]),
            "wqR": np.ascontiguousarray(wslice[("wq", g)][:, 1:]),
            "xqT": xT[("q", b)], "xkT": xT[("k", b)], "xvT": xT[("v", b)],
            "wq": wslice[("wq", g)], "wk": wslice[("wk", g)],
            "wv": wslice[("wv", g)], "wo": wslice[("wo", g)],
            "bq": wslice[("bq", g)], "bk": wslice[("bk", g)],
            "bv": wslice[("bv", g)],
        })

    res = run_bass_kernel_spmd(nc, in_maps, list(range(N_CORES)), trace=TRACE)
    LAST_RESULTS = res

    bo32 = np.asarray(bo, np.float32)
    out = np.empty((B, L, D), np.float32)
    for b in range(B):
        out[b] = res.results[2 * b]["outp"] + res.results[2 * b + 1]["outp"] + bo32
    return out


# revision 38
# speedup vs baseline: 1.0298x; 1.0204x over previous
"""Causal multi-head attention layer on 8 trn2 NeuronCores.

Sharding: 8 cores = 4 batches x 2 head-groups. Core c handles batch c//2 and
heads [8*(c%2), 8*(c%2)+8). Each core runs QKV projections for its 512-wide
head slice, causal flash attention for 8 heads, and a partial output
projection (its 512 rows of Wo). Host sums the two partials per batch + bo.

Schedule: the attention inner loop is PE-bound only if the exp-ACTIVATE
latency is hidden; projection/out-projection matmuls are flattened into
single-matmul "atoms" and woven one per attention step (plus bulk drains at
head-pair boundaries). outproj(1)/(2) are deferred into the last l-chunk,
which has no next-projection to weave.

Problem constants (hardcoded per contract): B=4, L=2048, D=1024, H=16, DK=DV=64.
"""

import sys

import os
for _p in ("/opt/trn_rl_repo", "/root/.axon_site/_ro/trn_rl_repo"):
    if os.path.isdir(_p) and _p not in sys.path:
        sys.path.insert(0, _p)

import numpy as np
import ml_dtypes

import concourse.bass as bass
import concourse.tile as tile
from concourse import bacc, mybir
from concourse.bass_utils import run_bass_kernel_spmd
BF16 = ml_dtypes.bfloat16

B, L, D, H, DK, DV = 4, 2048, 1024, 16, 64, 64
N_CORES = 8
HL = 8          # heads per core
DH = 512        # local head dim (HL * DK)
P = 128
LC = 512        # l-chunk
NLC = L // LC   # 4
NDC = D // P    # 8 contraction chunks for projections
NKC = DH // P   # 4 dk chunks
NST = L // P    # 16 s tiles
VW = DV + 1     # 65: V columns + ones column
SCALE = 1.0 / np.sqrt(DK)
MASK_NEG = -1.0e5

TRACE = False          # set by test harness for profiling runs
LAST_RESULTS = None    # BassKernelResults of the last run (for profiling)

_COMPILED = None


def _build():
    f32 = mybir.dt.float32
    bf16 = mybir.dt.bfloat16
    AF = mybir.ActivationFunctionType

    nc = bacc.Bacc("TRN2", target_bir_lowering=False, debug=False,
                   num_devices=N_CORES)

    xq0 = nc.dram_tensor("xq0", [P, LC], bf16, kind="ExternalInput").ap()
    wq0 = nc.dram_tensor("wq0", [P, DH], bf16, kind="ExternalInput").ap()
    xqR = nc.dram_tensor("xqR", [P, NDC - 1, LC], bf16, kind="ExternalInput").ap()
    wqR = nc.dram_tensor("wqR", [P, NDC - 1, DH], bf16, kind="ExternalInput").ap()
    xqT = nc.dram_tensor("xqT", [NLC, P, NDC, LC], bf16, kind="ExternalInput").ap()
    xkT = nc.dram_tensor("xkT", [NLC, P, NDC, LC], bf16, kind="ExternalInput").ap()
    xvT = nc.dram_tensor("xvT", [NLC, P, NDC, LC], bf16, kind="ExternalInput").ap()
    wq = nc.dram_tensor("wq", [P, NDC, DH], bf16, kind="ExternalInput").ap()
    wk = nc.dram_tensor("wk", [P, NDC, DH], bf16, kind="ExternalInput").ap()
    wv = nc.dram_tensor("wv", [P, NDC, DH], bf16, kind="ExternalInput").ap()
    wo = nc.dram_tensor("wo", [P, NKC, D], bf16, kind="ExternalInput").ap()
    bq = nc.dram_tensor("bq", [P, NKC], f32, kind="ExternalInput").ap()
    bk = nc.dram_tensor("bk", [P, NKC], f32, kind="ExternalInput").ap()
    bv = nc.dram_tensor("bv", [P, DH], f32, kind="ExternalInput").ap()
    outp = nc.dram_tensor("outp", [L, D], f32, kind="ExternalOutput").ap()

    from contextlib import ExitStack

    with tile.TileContext(nc) as tc, ExitStack() as ctx:
        const = ctx.enter_context(tc.tile_pool(name="const", bufs=1))
        kvp = ctx.enter_context(tc.tile_pool(name="kv", bufs=1))
        xp = ctx.enter_context(tc.tile_pool(name="x", bufs=2))
        qp = ctx.enter_context(tc.tile_pool(name="qt", bufs=2))
        ptp = ctx.enter_context(tc.tile_pool(name="pt", bufs=6))
        atp = ctx.enter_context(tc.tile_pool(name="at", bufs=4))
        osb = ctx.enter_context(tc.tile_pool(name="osb", bufs=5))
        nrm = ctx.enter_context(tc.tile_pool(name="nrm", bufs=3))
        ps_pj = ctx.enter_context(tc.tile_pool(name="ps_pj", bufs=2, space="PSUM"))
        ps_s = ctx.enter_context(tc.tile_pool(name="ps_s", bufs=2, space="PSUM"))
        ps_o = ctx.enter_context(tc.tile_pool(name="ps_o", bufs=2, space="PSUM"))

        # ---- constants / first-chunk loads ----
        # Startup is DMA-issue bound (~550ns per dma_start): split the first
        # matmul's deps (wq dc=0 + xq dc=0) onto Sync, the remainders onto
        # the otherwise-idle Scalar/Vector queues, k/v path onto GpSimd.
        # first-chunk deps live in their own tiles: a tile's readers wait on
        # ALL of its writer DMAs, so splitting one tile's load would not help
        wqa = const.tile([P, DH], bf16, tag="wqa")
        wq_sb = const.tile([P, NDC - 1, DH], bf16, tag="wq")
        x0a = const.tile([P, LC], bf16, tag="xq0")
        x0r = const.tile([P, NDC - 1, LC], bf16, tag="xq0r")
        x0 = [None] + [xp.tile([P, NDC, LC], bf16, tag=nm, name=nm)
                       for nm in ("xk", "xv")]
        nc.sync.dma_start(wqa[:], wq0[:])
        nc.sync.dma_start(x0a[:], xq0[:])
        nc.scalar.dma_start(wq_sb[:], wqR[:])
        nc.scalar.dma_start(x0r[:], xqR[:])

        # additive causal mask for diagonal 128x128 blocks of S^T (s part, l free)
        cmask = const.tile([P, P], f32, tag="cmask")
        nc.gpsimd.memset(cmask[:], 0.0)
        nc.gpsimd.affine_select(
            out=cmask[:], in_=cmask[:],
            compare_op=mybir.AluOpType.is_ge,
            fill=MASK_NEG, base=0,
            pattern=[[1, P]], channel_multiplier=-1,
        )
        nc.gpsimd.dma_start(x0[1][:], xkT[0])
        nc.gpsimd.dma_start(x0[2][:], xvT[0])
        wk_sb = const.tile([P, NDC, DH], bf16, tag="wk")
        nc.gpsimd.dma_start(wk_sb[:], wk[:])
        wv_sb = const.tile([P, NDC, DH], bf16, tag="wv")
        nc.gpsimd.dma_start(wv_sb[:], wv[:])
        bq_sb = const.tile([P, NKC], f32, tag="bq")
        nc.sync.dma_start(bq_sb[:], bq[:])
        bk_sb = const.tile([P, NKC], f32, tag="bk")
        nc.sync.dma_start(bk_sb[:], bk[:])
        bv_sb = const.tile([P, DH], f32, tag="bv")
        nc.sync.dma_start(bv_sb[:], bv[:])
        wo_sb = const.tile([P, NKC, D], bf16, tag="wo")
        nc.sync.dma_start(wo_sb[:], wo[:])

        # persistent K^T (dk, s) and V (s, dv|1) for the whole core
        kT_sb = kvp.tile([P, NKC, L], bf16, tag="kT")
        v_sb = kvp.tile([P, NST, HL * VW], bf16, tag="v")
        ones_view = v_sb[:].rearrange("p t (h c) -> p t h c", c=VW)[:, :, :, DV:]
        nc.vector.memset(ones_view, 1.0)

        def wqf(dc):
            return wqa[:] if dc == 0 else wq_sb[:, dc - 1, :]
        W = {"q": (wqf, bq_sb), "k": (lambda dc: wk_sb[:, dc, :], bk_sb)}

        def load_x(lc, preloaded=None):
            if preloaded is not None:
                xk_t, xv_t = preloaded[1], preloaded[2]
                return (lambda dc: x0a[:] if dc == 0 else x0r[:, dc - 1, :],
                        lambda dc: xk_t[:, dc, :],
                        lambda dc: xv_t[:, dc, :])
            xs = []
            for nm, dram in (("xq", xqT), ("xk", xkT), ("xv", xvT)):
                t = xp.tile([P, NDC, LC], bf16, tag=nm, name=nm)
                nc.sync.dma_start(t[:], dram[lc])
                xs.append(t)
            return tuple((lambda t: lambda dc: t[:, dc, :])(t) for t in xs)

        # ---- atom builders: each atom is ~one PE matmul (plus cheap tail) ----
        def proj_atoms(lc, xs, qt_t):
            """QKV projection for chunk lc as a flat list of matmul atoms."""
            xq_f, xk_f, xv_f = xs
            atoms = []

            def qk_group(which, kc):
                w_f, b_sb = W[which]
                x_f = xq_f if which == "q" else xk_f
                cell = {}

                def mk(dc):
                    def emit():
                        if dc == 0:
                            cell["ps"] = ps_pj.tile([P, LC], f32, tag="ps_pj",
                                                    name="ps_pj")
                        nc.tensor.matmul(cell["ps"][:],
                                         w_f(dc)[:, bass.ts(kc, P)],
                                         x_f(dc),
                                         start=(dc == 0), stop=(dc == NDC - 1))
                        if dc == NDC - 1:
                            dst = (qt_t[:, kc, :] if which == "q"
                                   else kT_sb[:, kc, bass.ts(lc, LC)])
                            nc.vector.tensor_scalar_add(dst, cell["ps"][:],
                                                        b_sb[:, kc:kc + 1])
                    return emit
                return [mk(dc) for dc in range(NDC)]

            def v_group(j):
                st = lc * (LC // P) + j
                cell = {}

                def mk(dc):
                    def emit():
                        if dc == 0:
                            cell["ps"] = ps_pj.tile([P, LC], f32, tag="ps_pj",
                                                    name="ps_pj")
                        nc.tensor.matmul(cell["ps"][:],
                                         xv_f(dc)[:, bass.ts(j, P)],
                                         wv_sb[:, dc, :],
                                         start=(dc == 0), stop=(dc == NDC - 1))
                        if dc == NDC - 1:
                            vv = v_sb[:, st, :].rearrange(
                                "p (h c) -> p h c", c=VW)[:, :, :DV]
                            nc.vector.tensor_tensor(
                                vv,
                                cell["ps"][:].rearrange("p (h c) -> p h c", c=DV),
                                bv_sb[:].rearrange("p (h c) -> p h c", c=DV),
                                mybir.AluOpType.add)
                    return emit
                return [mk(dc) for dc in range(NDC)]

            for kc in range(NKC):
                atoms += qk_group("q", kc)
            for kc in range(NKC):
                atoms += qk_group("k", kc)
            for j in range(LC // P):
                atoms += v_group(j)
            return [(0, a) for a in atoms]

        def outproj_atoms(lc, at_t):
            """Out-projection for chunk lc as (ready_slot, fn) matmul atoms."""
            atoms = []

            def op_group(lt):
                o_cell = {}
                group = []
                for n in range(2):
                    cell = {}

                    def mk(hc, n=n, cell=cell):
                        def emit():
                            if hc == 0:
                                if "o" not in o_cell:
                                    o_cell["o"] = osb.tile([P, D], f32,
                                                           tag="o_sb", name="o_sb")
                                cell["ps"] = ps_pj.tile([P, LC], f32,
                                                        tag="ps_pj", name="ps_pj")
                            nc.tensor.matmul(cell["ps"][:],
                                             at_t[hc][:, bass.ts(lt, P)],
                                             wo_sb[:, hc, bass.ts(n, 512)],
                                             start=(hc == 0), stop=(hc == NKC - 1))
                            if hc == NKC - 1:
                                nc.vector.tensor_copy(
                                    o_cell["o"][:, bass.ts(n, 512)], cell["ps"][:])
                        return emit
                    group += [(0, mk(hc)) for hc in range(NKC)]

                def flush():
                    nc.sync.dma_start(
                        outp[lc * LC + lt * P: lc * LC + (lt + 1) * P, :],
                        o_cell["o"][:])
                group.append((0, flush))
                return group

            for lt in range(LC // P):
                atoms += op_group(lt)
            return atoms

        def outproj_split_atoms(lc, at_t, n_st):
            """Out-projection for the chunk whose attention is still running:
            two-hc psum chains gated by ready_slot so each matmul emits only
            after the normalize that produces its at_t half. Tail = only the
            hc2+hc3 chains' last matmuls + adds."""
            o_cells = [{} for _ in range(LC // P)]
            halves = []
            # chain groups: (hc0+hc1 -> copy) ready when at1 lands, then
            # single-hc add chains for hc2 and hc3; whole-chain ready values
            # so FIFO gating never leaves a psum chain half-open
            for hcs in ((0, 1), (2,), (3,)):
                ready = (hcs[-1] + 1) * n_st
                for lt in range(LC // P):
                    for n in range(2):
                        cell = {}
                        for j, hc in enumerate(hcs):
                            last = j == len(hcs) - 1

                            def emit(j=j, hc=hc, n=n, lt=lt, cell=cell,
                                     first=(hcs[0] == 0), last=last):
                                if j == 0:
                                    if "o" not in o_cells[lt]:
                                        o_cells[lt]["o"] = osb.tile(
                                            [P, D], f32, tag="o_sb", name="o_sb")
                                    cell["ps"] = ps_pj.tile([P, LC], f32,
                                                            tag="ps_pj", name="ps_pj")
                                nc.tensor.matmul(cell["ps"][:],
                                                 at_t[hc][:, bass.ts(lt, P)],
                                                 wo_sb[:, hc, bass.ts(n, 512)],
                                                 start=(j == 0), stop=last)
                                if last:
                                    dst = o_cells[lt]["o"][:, bass.ts(n, 512)]
                                    if first:
                                        nc.vector.tensor_copy(dst, cell["ps"][:])
                                    else:
                                        nc.vector.tensor_tensor(
                                            dst, dst, cell["ps"][:],
                                            mybir.AluOpType.add)
                            halves.append((ready, emit))

            def mk_flush(lt):
                def flush():
                    nc.sync.dma_start(
                        outp[lc * LC + lt * P: lc * LC + (lt + 1) * P, :],
                        o_cells[lt]["o"][:])
                return flush
            for lt in range(LC // P):
                halves.append((4 * n_st, mk_flush(lt)))
            return halves

        def attention(lc, qt_t, atoms, at_t):
            """Causal attention for chunk lc as a flat (pair, step) pipeline:
            scores+exp issue one slot ahead (across pair boundaries, so the
            scalar engine stays fed through the normalize/drain regions); one
            filler atom weaves per slot when its ready_slot allows, surplus
            drains at pair boundaries."""
            n_st = (lc + 1) * (LC // P)

            def mm1(hp, st):
                # S^T: two heads packed on PE row halves, one 2-bank psum
                jj = st - lc * (LC // P)
                nc0 = jj * P if jj >= 0 else 0
                s01 = ps_s.tile([P, 2 * LC], f32, tag="ps_s", name="ps_s")
                nc.tensor.matmul(s01[:, nc0:LC], kT_sb[0:64, hp, bass.ts(st, P)],
                                 qt_t[0:64, hp, nc0:], start=True, stop=True,
                                 tile_position=(0, 0))
                nc.tensor.matmul(s01[:, LC + nc0:], kT_sb[64:128, hp, bass.ts(st, P)],
                                 qt_t[64:128, hp, nc0:], start=True, stop=True,
                                 tile_position=(64, 0))
                if jj >= 0:
                    dview = s01[:].rearrange("p (t c) -> p t c", t=2)[:, :, nc0:nc0 + P]
                    nc.vector.tensor_tensor(
                        dview, dview,
                        cmask[:, None, :].to_broadcast([P, 2, P]),
                        mybir.AluOpType.add)
                return s01, nc0

            def normalize(hp, po0, po1):
                # evict psum, then normalize off-psum (h1 first: longer chain)
                un1 = nrm.tile([P, LC], f32, tag="un", name="un")
                nc.vector.tensor_copy(un1[0:VW, :], po1[0:VW, :])
                un0 = nrm.tile([P, LC], f32, tag="un", name="un")
                nc.vector.tensor_copy(un0[0:VW, :], po0[0:VW, :])

                rz1 = nrm.tile([1, LC], f32, tag="rz", name="rz")
                nc.sync.dma_start(rz1[:], un1[64:65, :])
                rz0 = nrm.tile([1, LC], f32, tag="rz", name="rz")
                nc.sync.dma_start(rz0[:], un0[64:65, :])
                rr1 = nrm.tile([1, LC], f32, tag="rr", name="rr")
                nc.vector.reciprocal_approx_fast(rr1[:], rz1[:])
                rr0 = nrm.tile([1, LC], f32, tag="rr", name="rr")
                nc.vector.reciprocal_approx_fast(rr0[:], rz0[:])
                rb1 = nrm.tile([64, LC], f32, tag="rb", name="rb")
                nc.gpsimd.partition_broadcast(rb1[:], rr1[:])
                rb0 = nrm.tile([64, LC], f32, tag="rb", name="rb")
                nc.gpsimd.partition_broadcast(rb0[:], rr0[:])
                tmp1 = nrm.tile([64, LC], bf16, tag="tmp1", name="tmp1")
                nc.vector.tensor_mul(tmp1[:], un1[0:64, :], rb1[:])
                nc.sync.dma_start(at_t[hp][64:128, :], tmp1[:])
                nc.vector.tensor_mul(at_t[hp][0:64, :], un0[0:64, :], rb0[:])

            slots = [(hp, st) for hp in range(NKC) for st in range(n_st)]
            po = {}
            pend = mm1(*slots[0])
            for i, (hp, st) in enumerate(slots):
                s01, nc0 = pend
                if i + 1 < len(slots):
                    pend = mm1(*slots[i + 1])  # PE one slot ahead of ACT
                pt01 = ptp.tile([P, 2 * LC], bf16, tag="pt", name="pt")
                nc.scalar.activation(
                    pt01[:].rearrange("p (t c) -> p t c", t=2)[:, :, nc0:],
                    s01[:].rearrange("p (t c) -> p t c", t=2)[:, :, nc0:],
                    AF.Exp, bias=0.0, scale=float(SCALE))
                if atoms and atoms[0][0] <= i:
                    atoms.pop(0)[1]()  # PE filler for the ACT latency
                if st == 0:
                    po[hp] = (ps_o.tile([P, LC], f32, tag="ps_o", name="ps_o"),
                              ps_o.tile([P, LC], f32, tag="ps_o", name="ps_o"))
                po0, po1 = po[hp]
                h0, h1 = 2 * hp, 2 * hp + 1
                nc.tensor.matmul(po0[0:VW, nc0:], v_sb[:, st, h0 * VW:(h0 + 1) * VW],
                                 pt01[:, nc0:LC],
                                 start=(st == 0), stop=(st == n_st - 1))
                nc.tensor.matmul(po1[0:VW, nc0:], v_sb[:, st, h1 * VW:(h1 + 1) * VW],
                                 pt01[:, LC + nc0:],
                                 start=(st == 0), stop=(st == n_st - 1))
                if st == n_st - 1:
                    normalize(hp, po0, po1)
                    # bulk-drain surplus atoms, keeping a reserve to cover
                    # the final pair's normalize latency with PE work
                    slots_left = len(slots) - 1 - i
                    bounds_left = (slots_left + n_st - 1) // n_st
                    if bounds_left:
                        surplus = len(atoms) - slots_left - 18
                        b = -(-surplus // bounds_left) if surplus > 0 else 0
                        for _ in range(b):
                            if not atoms or atoms[0][0] > i:
                                break
                            atoms.pop(0)[1]()
            while atoms:
                atoms.pop(0)[1]()
            return at_t

        # ---- pipeline ----
        # proj(0) runs alone; attention(lc) weaves: lc=0 -> proj(1),
        # lc=1 -> proj(2)+outproj(0), lc=2 -> proj(3), lc=3 -> outproj(1)+(2)
        # +outproj(3) (split chains gated by ready_slot, so only the hc2+hc3
        # chains' final matmuls trail the last normalize).
        xs = load_x(0, preloaded=x0)
        qt0 = qp.tile([P, NKC, LC], bf16, tag="qt", name="qt")
        for _, a in proj_atoms(0, xs, qt0):
            a()
        qt_cur = qt0
        at_all = []
        for lc in range(NLC):
            at_t = [atp.tile([P, LC], bf16, tag=f"at{hc}", name=f"at{hc}")
                    for hc in range(NKC)]
            at_all.append(at_t)
            atoms = []
            if lc + 1 < NLC:
                xs = load_x(lc + 1)
                qt_nxt = qp.tile([P, NKC, LC], bf16, tag="qt", name="qt")
                atoms += proj_atoms(lc + 1, xs, qt_nxt)
            if lc == 1:
                atoms += outproj_atoms(0, at_all[0])
            if lc == 3:
                n_st3 = (lc + 1) * (LC // P)
                a2 = outproj_atoms(2, at_all[2])
                sp = outproj_split_atoms(3, at_t, n_st3)
                # hold 3 outproj(2) groups for the final drain, placed before
                # the at3-gated section: they execute on the PE while the last
                # pair's normalize chain runs, instead of idling behind it
                reserve = [(4 * n_st3, fn) for _, fn in a2[9:]]
                gated = [a for a in sp if a[0] > 3 * n_st3]
                atoms += outproj_atoms(1, at_all[1])
                atoms += a2[:9]
                atoms += [a for a in sp if a[0] <= 3 * n_st3]
                atoms += reserve + gated
            attention(lc, qt_cur, atoms, at_t)
            if lc + 1 < NLC:
                qt_cur = qt_nxt

    nc.compile()
    return nc


def _get_compiled():
    global _COMPILED
    if _COMPILED is None:
        _COMPILED = _build()
    return _COMPILED


def kernel(queries, keys, values, Wq, bq, Wk, bk, Wv, bv, Wo, bo):
    global LAST_RESULTS
    nc = _get_compiled()

    queries = np.asarray(queries, np.float32)
    keys = np.asarray(keys, np.float32)
    values = np.asarray(values, np.float32)

    def pack_x(x):
        # (L, D) -> (NLC, P, NDC, LC): [lc, p, dc, l] = x[lc*LC+l, dc*P+p]
        t = x.T.reshape(NDC, P, NLC, LC)          # [dc, p, lc, l]
        return np.ascontiguousarray(t.transpose(2, 1, 0, 3)).astype(BF16)

    xT = {}
    for b in range(B):
        xT[("q", b)] = pack_x(np.asarray(queries[b]))
        xT[("k", b)] = pack_x(np.asarray(keys[b]))
        xT[("v", b)] = pack_x(np.asarray(values[b]))

    wslice = {}
    for g in range(2):
        sl = slice(DH * g, DH * (g + 1))
        def pack_w(w):
            # (D, DH) -> (P, NDC, DH)
            return np.ascontiguousarray(
                w.reshape(NDC, P, DH).transpose(1, 0, 2)).astype(BF16)

        wslice[("wq", g)] = pack_w(np.asarray(Wq, np.float32)[:, sl])
        wslice[("wk", g)] = pack_w(np.asarray(Wk, np.float32)[:, sl])
        wslice[("wv", g)] = pack_w(np.asarray(Wv, np.float32)[:, sl])
        wslice[("wo", g)] = np.ascontiguousarray(
            np.asarray(Wo, np.float32)[sl, :].reshape(NKC, P, D).transpose(1, 0, 2)
        ).astype(BF16)
        wslice[("bq", g)] = np.ascontiguousarray(
            np.asarray(bq, np.float32)[sl].reshape(NKC, P).T)
        wslice[("bk", g)] = np.ascontiguousarray(
            np.asarray(bk, np.float32)[sl].reshape(NKC, P).T)
        wslice[("bv", g)] = np.ascontiguousarray(
            np.broadcast_to(np.asarray(bv, np.float32)[sl], (P, DH)))

    in_maps = []
    for c in range(N_CORES):
        b, g = c // 2, c % 2
        in_maps.append({
            "xq0": np.ascontiguousarray(xT[("q", b)][0, :, 0, :]),
            "wq0": np.ascontiguousarray(wslice[("wq", g)][:, 0]),
            "xqR": np.ascontiguousarray(xT[("q", b)][0, :, 1:, :]),
            "wqR": np.ascontiguousarray(wslice[("wq", g)][:, 1:]),
            "xqT": xT[("q", b)], "xkT": xT[("k", b)], "xvT": xT[("v", b)],
            "wq": wslice[("wq", g)], "wk": wslice[("wk", g)],
            "wv": wslice[("wv", g)], "wo": wslice[("wo", g)],
            "bq": wslice[("bq", g)], "bk": wslice[("bk", g)],
            "bv": wslice[("bv", g)],
        })

    res = run_bass_kernel_spmd(nc, in_maps, list(range(N_CORES)), trace=TRACE)
    LAST_RESULTS = res

    bo32 = np.asarray(bo, np.float32)
    out = np.empty((B, L, D), np.float32)
    for b in range(B):
        out[b] = res.results[2 * b]["outp"] + res.results[2 * b + 1]["outp"] + bo32
    return out
